# revision 4
# baseline (speedup 1.0000x reference)
"""GCN 2-layer message passing on 8 Trainium2 NeuronCores — v3.

Aggregate-then-transform: out_l = relu(c . (A+I)^T (c . h) @ W_l + b_l)
with c = rsqrt(deg). Since GCN is linear, the dense transform moves AFTER
aggregation, so the per-edge gather reads pre-scaled feature rows directly
and the two table launches of v2 disappear entirely:

- 2 launches (one shared NEFF): L1 gathers xc = c.x rows, aggregates per
  dst block via one-hot matmuls (+ self identity), epilogue per block:
  t1 = psum . c[d] (DVE) -> @W + bias-slot K=1 matmul (PE, out [d, f]
  row-major) -> ACT relu with per-partition scale (relu(x)*c = relu(c*x))
  producing h1c rows f32; host casts/concats -> L2 same program, scale=1.
- rsqrt(deg) is topology-only -> host input (kills the 80us DVE
  reciprocal + ACT sqrt prologue of v2).
- Edge layout machinery (spans/cells/stripes, idx wrap, dstl) unchanged
  from v2: edges partitioned by dst shard, grouped per (span of 4 dst
  blocks, src chunk), padded to the cross-core max per (block, chunk).
- Gathers: SWDGE dma_gather, 4 queues, <=64-stripe calls.
- Scatter: per-stripe one-hot matmuls accumulated in PSUM [f, d].
"""

import os
import numpy as np

import concourse.bass as bass
import concourse.tile as tile
from concourse import bacc, mybir
from concourse.bass_utils import run_bass_kernel_spmd

N = 100000
D = 128
NC = 8
SHARD = 12500
NBLK = 98              # 128-dst blocks per core (12544 padded)
PADN = NBLK * 128
CHUNK = 25000
NCHUNK = 4
SPAN = 4               # dst blocks per span
CALL_STR = 8           # stripes per gather call
NQ = 4
SP = True              # single_packet

_f32 = mybir.dt.float32
_bf16 = mybir.dt.bfloat16
_i16 = mybir.dt.int16

BENCH = bool(int(os.environ.get("KERNEL_BENCH", "0")))
EXEC_NS = []


def build_mp_nc(layout):
    """One GCN layer: gather prescaled rows, aggregate, transform, relu.

    Inputs: tab [N, D] bf16 (full prescaled table), tabself [PADN, D] bf16
    (own-shard rows), idxt/dstl (edge layout), disrep [128, PADN] f32
    (c replicated across partitions), sccol [128, NBLK] f32 (ACT scale
    column: c for layer 1, ones for layer 2), W [D, D] bf16, brow [1, D]
    bf16, idmat [128, 128] bf16. Output outR [PADN, D] f32 rows.
    """
    nc = bacc.Bacc("TRN2", target_bir_lowering=False, debug=False,
                   enable_asserts=False, num_devices=NC, num_swdge_queues=NQ)
    tab = nc.dram_tensor("tab", [N, D], _bf16, kind="ExternalInput").ap()
    tabself = nc.dram_tensor("tabself", [PADN, D], _bf16,
                             kind="ExternalInput").ap()
    idxt = nc.dram_tensor("idxt", [128, layout["idxw"]], _i16,
                          kind="ExternalInput").ap()
    dstl = nc.dram_tensor("dstl", [128, layout["Gtot"]], _bf16,
                          kind="ExternalInput").ap()
    disrep = nc.dram_tensor("disrep", [128, PADN], _f32,
                            kind="ExternalInput").ap()
    sccol = nc.dram_tensor("sccol", [128, NBLK], _f32,
                           kind="ExternalInput").ap()
    W = nc.dram_tensor("W", [D, D], _bf16, kind="ExternalInput").ap()
    brow = nc.dram_tensor("brow", [1, D], _bf16, kind="ExternalInput").ap()
    idmat = nc.dram_tensor("idmat", [128, 128], _bf16,
                           kind="ExternalInput").ap()
    outR = nc.dram_tensor("outR", [PADN, D], _f32, kind="ExternalOutput").ap()

    qi = 0
    with tile.TileContext(nc) as tc:
        with tc.tile_pool(name="sing", bufs=1) as sing, \
             tc.tile_pool(name="mp", bufs=3) as mp, \
             tc.tile_pool(name="sp", bufs=2) as spool, \
             tc.tile_pool(name="st", bufs=6) as stp, \
             tc.tile_pool(name="ip", bufs=3) as ip, \
             tc.tile_pool(name="ep", bufs=4) as ep, \
             tc.tile_pool(name="ps", bufs=4, space="PSUM") as ps:
            # c = rsqrt(deg), host-computed: replicated row-broadcast copy
            dis = sing.tile([128, PADN], _f32)
            DC = PADN // 4
            for jc in range(4):
                eng = nc.scalar if jc % 2 else nc.sync
                eng.dma_start(out=dis[:, jc * DC:(jc + 1) * DC],
                              in_=disrep[:, jc * DC:(jc + 1) * DC])
            scc = sing.tile([128, NBLK], _f32)
            nc.sync.dma_start(out=scc[:], in_=sccol[:])
            w = sing.tile([128, D], _bf16)
            nc.sync.dma_start(out=w[:], in_=W[:])
            bs = sing.tile([1, D], _bf16)
            nc.sync.dma_start(out=bs[:], in_=brow[:])
            ones1 = sing.tile([1, 128], _bf16)
            nc.vector.memset(ones1[:], 1.0)
            # iota_ext [128, 256] bf16: values 0..255 along free dim
            iota_i = sing.tile([128, 256], _i16)
            nc.gpsimd.iota(iota_i[:], pattern=[[1, 256]], base=0,
                           channel_multiplier=0)
            iota_b = sing.tile([128, 256], _bf16)
            nc.vector.tensor_copy(iota_b[:], iota_i[:])
            # identity [128, 128] bf16 from host (exact)
            ident = sing.tile([128, 128], _bf16)
            nc.scalar.dma_start(out=ident[:], in_=idmat[:])
            # whole tabself resident: tself[p, b, :] = tabself[b*128+p, :]
            tself = sing.tile([128, NBLK, D], _bf16)
            nc.scalar.dma_start(
                out=tself[:],
                in_=bass.AP(tensor=tabself.tensor, offset=tabself.offset,
                            ap=[[D, 128], [128 * D, NBLK], [1, D]]))
            # dstl for all stripes, loaded once
            dst_all = sing.tile([128, layout["Gtot"]], _bf16)
            nc.scalar.dma_start(out=dst_all[:], in_=dstl[:])

            for span in layout["spans"]:
                nstr_span = span["nstr_span"]
                g_base = span["g_base"]
                # gather all chunk groups of this span; one idx load/span
                m = mp.tile([128, nstr_span, 128], _bf16, tag="m")
                it = ip.tile([128, nstr_span * 8], _i16, tag="it")
                span_idx0 = span["chunks"][0]["idx_off"]
                nc.scalar.dma_start(
                    out=it[:],
                    in_=idxt[:, span_idx0:span_idx0 + nstr_span * 8])
                for cg in span["chunks"]:
                    c = cg["chunk"]
                    for (go, nsc) in cg["calls"]:
                        g = cg["g0"] + go
                        nc.gpsimd.dma_gather(
                            out_ap=m[:, g:g + nsc, :],
                            in_ap=tab[c * CHUNK:min((c + 1) * CHUNK, N), :],
                            idxs_ap=it[:, g * 8:(g + nsc) * 8],
                            num_idxs=nsc * 128,
                            num_idxs_reg=nsc * 128,
                            elem_size=D,
                            single_packet=SP,
                            queue_num=qi % NQ,
                        )
                        qi += 1
                # primary one-hot for the whole span: [128, nstr_span, 128]
                s = spool.tile([128, nstr_span, 128], _bf16, tag="s")
                dcol = dst_all[:, g_base:g_base + nstr_span]
                dap = dcol.ap
                nc.vector.tensor_tensor(
                    out=s[:],
                    in0=bass.AP(tensor=dst_all.tensor, offset=dcol.offset,
                                ap=[[dap[0][0], 128], [dap[1][0], nstr_span],
                                    [0, 128]]),
                    in1=bass.AP(tensor=iota_b.tensor, offset=iota_b[:].offset,
                                ap=[[iota_b[:].ap[0][0], 128], [0, nstr_span],
                                    [1, 128]]),
                    op=mybir.AluOpType.is_equal)
                # straddle one-hots: compare vs iota+128
                stiles = {}
                for bi, pairs in enumerate(span["pairs"]):
                    for (g, kind) in pairs:
                        if kind == 1:
                            st = stp.tile([128, 128], _bf16, tag="st")
                            gc = dst_all[:, g_base + g:g_base + g + 1]
                            nc.vector.tensor_tensor(
                                out=st[:],
                                in0=bass.AP(tensor=dst_all.tensor,
                                            offset=gc.offset,
                                            ap=[[dap[0][0], 128], [0, 128]]),
                                in1=iota_b[:, 128:256],
                                op=mybir.AluOpType.is_equal)
                            stiles[(bi, g)] = st
                # per block: aggregate (self + edge stripes) -> epilogue
                nb = len(span["blocks"])
                ob = ep.tile([128, SPAN, D], _f32, tag="ob")
                for bi, pairs in enumerate(span["pairs"]):
                    b = span["blocks"][bi]
                    p1 = ps.tile([128, 128], _f32, space="PSUM")
                    nc.tensor.matmul(out=p1[:], lhsT=tself[:, b, :],
                                     rhs=ident[:],
                                     start=True, stop=(len(pairs) == 0))
                    for k, (g, kind) in enumerate(pairs):
                        rhs = s[:, g, :] if kind == 0 else stiles[(bi, g)][:]
                        nc.tensor.matmul(out=p1[:], lhsT=m[:, g, :], rhs=rhs,
                                         start=False,
                                         stop=(k == len(pairs) - 1))
                    # t1[f, d] = agg . c[d]
                    t1 = ep.tile([128, 128], _bf16, tag="t1")
                    nc.vector.tensor_tensor(
                        out=t1[:], in0=p1[:],
                        in1=dis[:, b * 128:(b + 1) * 128],
                        op=mybir.AluOpType.mult)
                    # p2[d, f'] = t1^T @ W + 1 (x) b
                    p2 = ps.tile([128, D], _f32, space="PSUM")
                    nc.tensor.matmul(out=p2[:], lhsT=t1[:], rhs=w[:],
                                     start=True, stop=False)
                    nc.tensor.matmul(out=p2[:], lhsT=ones1[:], rhs=bs[:],
                                     start=False, stop=True)
                    # relu(p2)*scc[d] == relu(p2*scc[d]) since scc > 0
                    nc.scalar.activation(ob[:, bi, :], p2[:],
                                         mybir.ActivationFunctionType.Relu,
                                         bias=0.0, scale=scc[:, b:b + 1])
                b0 = span["blocks"][0]
                nc.sync.dma_start(
                    out=bass.AP(tensor=outR.tensor,
                                offset=outR.offset + b0 * 128 * D,
                                ap=[[D, 128], [128 * D, nb], [1, D]]),
                    in_=ob[:, :nb, :])
    nc.compile()
    return nc


def prep_edges(edge_index):
    """Build the SPMD-uniform span/cell/stripe layout + per-core tensors.

    Self-loops are NOT in the gather stream — handled per block via a
    static DMA of the core's own table rows (tabself input). deg still
    counts them (+1)."""
    src = edge_index[0]
    dst = edge_index[1]
    deg = (np.bincount(dst, minlength=N) + 1).astype(np.float32)

    per_core = []
    cnts = np.zeros((NC, NBLK, NCHUNK), dtype=np.int64)
    for c in range(NC):
        sel = (dst >= c * SHARD) & (dst < (c + 1) * SHARD)
        s_ = src[sel]
        dl = dst[sel] - c * SHARD
        blk = dl // 128
        chk = s_ // CHUNK
        order = np.lexsort((dl, blk, chk))
        s_, dl, blk, chk = s_[order], dl[order], blk[order], chk[order]
        per_core.append((s_, dl, blk, chk))
        np.add.at(cnts[c], (blk, chk), 1)

    cellmax = cnts.max(axis=0)  # [NBLK, NCHUNK]

    spans = []
    g_abs = 0
    idx_off = 0
    for s0 in range(0, NBLK, SPAN):
        bl = list(range(s0, min(s0 + SPAN, NBLK)))
        chunks = []
        nstr_span = 0
        # cell slot geometry per chunk group
        pairs = [[] for _ in bl]
        for ch in range(NCHUNK):
            slots = int(sum(cellmax[b, ch] for b in bl))
            if slots == 0:
                continue
            nstr = (slots + 127) // 128
            # cell boundaries within the group
            bounds = []
            pos = 0
            for bi, b in enumerate(bl):
                bounds.append((pos, pos + int(cellmax[b, ch]), bi))
                pos += int(cellmax[b, ch])
            # stripe -> cell overlaps
            for g in range(nstr):
                lo, hi = g * 128, (g + 1) * 128
                prim = None
                for (a, bnd, bi) in bounds:
                    if a < hi and bnd > lo:  # overlap
                        if prim is None:
                            prim = bi
                            pairs[bi].append((nstr_span + g, 0))
                        else:
                            pairs[bi].append((nstr_span + g, 1))
                if prim is None:
                    pass  # tail-pad stripe, no block
            calls = []
            go = 0
            while go < nstr:
                nsc = min(CALL_STR, nstr - go)
                calls.append((go, nsc))
                go += nsc
            chunks.append({"chunk": ch, "g0": nstr_span, "nstr": nstr,
                           "idx_off": idx_off, "calls": calls,
                           "bounds": bounds, "slots": slots})
            idx_off += nstr * 8
            nstr_span += nstr
        spans.append({"blocks": bl, "chunks": chunks, "g_base": g_abs,
                      "nstr_span": nstr_span, "pairs": pairs})
        g_abs += nstr_span
    layout = {"spans": spans, "Gtot": g_abs, "idxw": idx_off}

    # per-core tensors
    datas = []
    for c in range(NC):
        s_, dl, blk, chk = per_core[c]
        # cell start offsets in the per-core sorted stream
        cell_n = np.zeros((NBLK, NCHUNK), dtype=np.int64)
        np.add.at(cell_n, (blk, chk), 1)
        cell_start = np.zeros(NBLK * NCHUNK + 1, dtype=np.int64)
        np.cumsum(cell_n.reshape(NBLK, NCHUNK).T.ravel(), out=cell_start[1:])
        # stream is sorted by (chk, blk, dl): cell (b, ch) starts at
        # cell_start[ch * NBLK + b] in (ch-major, blk-minor) order
        idx_arr = np.zeros((128, layout["idxw"]), dtype=np.int16)
        dstl_arr = np.full((layout["Gtot"], 128), 999.0, dtype=np.float32)
        for span in spans:
            bl = span["blocks"]
            for cg in span["chunks"]:
                ch = cg["chunk"]
                nstr = cg["nstr"]
                slot_idx = np.zeros(nstr * 128, dtype=np.int16)
                slot_dst = np.full(nstr * 128, 999.0, dtype=np.float32)
                for (a, bnd, bi) in cg["bounds"]:
                    b = bl[bi]
                    st = cell_start[ch * NBLK + b]
                    n_real = int(cell_n[b, ch])
                    # this core's real edges fill the cell front
                    sl = slice(a, a + n_real)
                    slot_idx[sl] = (s_[st:st + n_real] - ch * CHUNK).astype(
                        np.int16)
                    slot_dst[sl] = (dl[st:st + n_real] - 128 * b).astype(
                        np.float32)
                # adjust relative offsets: for each stripe, slots belonging
                # to a cell that is NOT the stripe's primary get +128
                for g in range(nstr):
                    lo, hi = g * 128, (g + 1) * 128
                    prim = None
                    for (a, bnd, bi) in cg["bounds"]:
                        if a < hi and bnd > lo:
                            if prim is None:
                                prim = bi
                            elif prim is not None:
                                sl = slice(max(a, lo), min(bnd, hi))
                                slot_dst[sl] += 128.0
                # write idx wrap + dstl columns
                for g in range(nstr):
                    eg = slot_idx[g * 128:(g + 1) * 128]
                    wr = eg.reshape(8, 16).T
                    col0 = cg["idx_off"] + g * 8
                    idx_arr[:, col0:col0 + 8] = np.tile(wr, (8, 1))
                    dstl_arr[span["g_base"] + cg["g0"] + g, :] = \
                        slot_dst[g * 128:(g + 1) * 128]
        datas.append({
            "idxt": idx_arr,
            "dstl": np.ascontiguousarray(
                dstl_arr.T).astype(np.dtype("bfloat16")),
        })
    return deg, layout, datas


_CACHE = {}


def kernel(x, edge_index, W1, b1, W2, b2):
    x = np.asarray(x)
    edge_index = np.asarray(edge_index).astype(np.int64)
    W1, b1 = np.asarray(W1), np.asarray(b1)
    W2, b2 = np.asarray(W2), np.asarray(b2)

    deg, layout, datas = prep_edges(edge_index)

    if "mp" not in _CACHE:
        _CACHE["mp"] = build_mp_nc(layout)

    core_ids = list(range(NC))
    bf16 = np.dtype("bfloat16")
    cvec = (1.0 / np.sqrt(deg)).astype(np.float32)  # [N]

    disrep = []
    sccol1 = []
    for c in range(NC):
        cpad = np.ones(PADN, dtype=np.float32)
        cpad[:SHARD] = cvec[c * SHARD:(c + 1) * SHARD]
        disrep.append(np.ascontiguousarray(
            np.broadcast_to(cpad, (128, PADN))))
        sccol1.append(np.ascontiguousarray(
            cpad.reshape(NBLK, 128).T).astype(np.float32))
    sccol2 = np.ones((128, NBLK), dtype=np.float32)

    def _selfslices(tab_full):
        out = []
        for c in range(NC):
            t = np.zeros((PADN, D), dtype=bf16)
            t[:SHARD] = tab_full[c * SHARD:(c + 1) * SHARD]
            out.append(t)
        return out

    def _run(in_maps, name):
        print(f"launch: {name}", flush=True)
        res = run_bass_kernel_spmd(_CACHE["mp"], in_maps, core_ids,
                                   trace=BENCH)
        if BENCH:
            EXEC_NS.append(res.exec_time_ns)
        return res

    idmat = np.eye(128, dtype=np.float32).astype(bf16)
    W1b = np.ascontiguousarray(W1).astype(bf16)
    W2b = np.ascontiguousarray(W2).astype(bf16)
    b1row = b1.reshape(1, D).astype(bf16)
    b2row = b2.reshape(1, D).astype(bf16)

    # L1: gather xc = c.x rows
    xc = (x * cvec[:, None]).astype(bf16)
    ts1 = _selfslices(xc)
    res = _run([{"tab": xc, "tabself": ts1[c], "idxt": datas[c]["idxt"],
                 "dstl": datas[c]["dstl"], "disrep": disrep[c],
                 "sccol": sccol1[c], "W": W1b, "brow": b1row,
                 "idmat": idmat} for c in core_ids], "mp1")
    h1c = np.concatenate(
        [np.asarray(res.results[c]["outR"])[:SHARD] for c in core_ids],
        axis=0).astype(bf16)

    # L2: gather h1c rows, scale=1 -> final h2
    ts2 = _selfslices(h1c)
    res = _run([{"tab": h1c, "tabself": ts2[c], "idxt": datas[c]["idxt"],
                 "dstl": datas[c]["dstl"], "disrep": disrep[c],
                 "sccol": sccol2, "W": W2b, "brow": b2row,
                 "idmat": idmat} for c in core_ids], "mp2")
    out = np.empty((N, D), dtype=np.float32)
    for c in range(NC):
        out[c * SHARD:(c + 1) * SHARD] = \
            np.asarray(res.results[c]["outR"])[:SHARD]
    return out
